# revision 13
# baseline (speedup 1.0000x reference)
"""Contrast-depth MSE loss on 8 Trainium2 NeuronCores.

Math: with d = out - label (per image, 32x32 grid flattened to p in [0,1024)),
the loss is an exact quadratic form

    loss = sum_{p,q} C[p,q] * G[p,q] / (B*8*30*30),
    G[p,q] = sum_img d[img,p] * d[img,q]

where C (the contrast-depth-conv quadratic form) is supported on the
diagonals q-p in {0, +-1, +-31, +-32, +-33}.  Each core computes banded
Gram blocks G[128k+r, 128k+c] (c in [0,161)) on the TensorEngine with
PSUM accumulation over its 2048-image shard; the host applies the C
weights to the diagonals and reduces across cores.

The host computes d = out - label exactly in fp32 and ships only d as
fp8e4m3 (2MB/core instead of 4.2MB) -- better numerics than the
quantize-then-subtract it replaces, half the DMA, and no on-device
vector work.  The PE runs in DoubleRow perf mode: each matmul contracts
two 128-image slots at once (lhsT/rhs carry 2 k-planes), which halves
both the instruction count and the weight-load traffic and doubles the
moving-stream rate.  Gram accumulation stays fp32 in PSUM.
"""

import numpy as np
import ml_dtypes

_B = 16384
_H = 32
_W = 32
_P = _H * _W  # 1024 pixels
_NCORES = 8
_BSH = _B // _NCORES  # 2048 images per core
_TILE = 128
_BAND = 161  # 128 + max diagonal offset (33)
_NSLOT = 16  # image-slots (128 images each)
_NPAIR = _NSLOT // 2
_NWARM = 20  # PE warmup matmuls (DVFS ramp) while DMA chunk 0 is in flight


def _block_ncols(k: int) -> int:
    return min(_BAND, _P - 128 * k)


_GRAM_COLS = sum(_block_ncols(k) for k in range(8))  # 7*161 + 128 = 1255


def _build_weights() -> np.ndarray:
    """[128, _GRAM_COLS] weights s.t. loss_sum = sum(W * gram_blocks)."""
    C = np.zeros((_P, _P), dtype=np.float64)
    offs = [(a, b) for a in range(3) for b in range(3) if (a, b) != (1, 1)]
    for a, b in offs:
        for i in range(_H - 2):
            for j in range(_W - 2):
                p = (i + a) * _W + (j + b)  # neighbor pixel
                q = (i + 1) * _W + (j + 1)  # center pixel
                C[p, p] += 1.0
                C[q, q] += 1.0
                C[p, q] -= 1.0
                C[q, p] -= 1.0
    W = np.zeros((_TILE, _GRAM_COLS), dtype=np.float64)
    off = 0
    for k in range(8):
        ncols = _block_ncols(k)
        for delta in (0, 1, 31, 32, 33):
            for r in range(_TILE):
                p = 128 * k + r
                q = p + delta
                c = r + delta
                if q >= _P or c >= ncols:
                    continue
                W[r, off + c] = C[p, q] * (1.0 if delta == 0 else 2.0)
        off += ncols
    return W


_WFULL = _build_weights()

_NC_CACHE = None


def _build_nc():
    import concourse.bacc as bacc
    import concourse.mybir as mybir
    import concourse.tile as tile

    nc = bacc.Bacc()
    d_d = nc.dram_tensor("d", [_BSH, _P], mybir.dt.float8e4, kind="ExternalInput")
    gram_d = nc.dram_tensor(
        "gram", [_TILE, _GRAM_COLS], mybir.dt.bfloat16, kind="ExternalOutput"
    )

    with tile.TileContext(nc) as tc:
        with (
            tc.tile_pool(name="buf", bufs=1) as buf_pool,
            tc.tile_pool(name="ps", bufs=1, space="PSUM") as psum_pool,
        ):
            grams = []
            offs = []
            off = 0
            for k in range(8):
                ncols = _block_ncols(k)
                grams.append(
                    psum_pool.tile(
                        [_TILE, ncols], mybir.dt.float32, tag=f"g{k}", name=f"g{k}"
                    )
                )
                offs.append(off)
                off += ncols

            # d[part, slot, pixel]: partition p of DMA chunk c holds images
            # 256c+2p (slot 2c) and 256c+2p+1 (slot 2c+1), 2KB contiguous.
            d = buf_pool.tile([_TILE, _NSLOT, _P], mybir.dt.float8e4, tag="d", name="d")
            result = buf_pool.tile(
                [_TILE, _GRAM_COLS], mybir.dt.bfloat16, tag="r", name="r"
            )
            dummy = buf_pool.tile(
                [_TILE, _BAND], mybir.dt.float8e4, tag="z", name="dummy"
            )

            # DMA triggers cost ~0.6us on the issuing queue and data lags the
            # trigger by ~1.5us (DGE->engine latency), so pair 0's two slots
            # are split into half-chunks triggered in parallel on sync and
            # gpsimd, and the remaining triggers alternate between the two
            # queues so no chunk ever gates the matmul stream.
            nc.gpsimd.memset(dummy[:], 0.0)
            issuers = [nc.sync, nc.gpsimd]
            for c in range(_NPAIR):
                n0 = c * 2 * _TILE
                issuers[c % 2].dma_start(
                    out=d[:, 2 * c : 2 * c + 2, :],
                    in_=d_d[n0 : n0 + 2 * _TILE, :],
                )

            # Warmup matmuls on a zeroed slab while the first DMA chunk is in
            # flight: keeps the PE busy so its DVFS p-state ramps toward
            # 2.4GHz before the real stream starts (cold PE issues at only
            # 1.2GHz).  Results land in g7, which pair 0 resets (start=True).
            for _ in range(_NWARM):
                nc.tensor.matmul(
                    grams[7][:, :128],
                    lhsT=dummy[:, 0:128],
                    rhs=dummy[:, 0:128],
                    start=True,
                    stop=True,
                )

            # one DoubleRow matmul per (pair, block): contracts both slots'
            # 128 images in a single instruction (2 k-planes).
            for c in range(_NPAIR):
                s0 = 2 * c
                for k in range(8):
                    ncols = _block_ncols(k)
                    nc.tensor.matmul(
                        grams[k][:, :ncols],
                        lhsT=d[:, s0 : s0 + 2, 128 * k : 128 * k + 128],
                        rhs=d[:, s0 : s0 + 2, 128 * k : 128 * k + ncols],
                        start=(c == 0),
                        stop=(c == _NPAIR - 1),
                        perf_mode=mybir.MatmulPerfMode.DoubleRow,
                    )

            # PSUM -> SBUF casts split across three engines (each block's copy
            # starts as soon as its stop-matmul retires), then one output DMA.
            for k in range(8):
                ncols = _block_ncols(k)
                dst = result[:, offs[k] : offs[k] + ncols]
                if k < 4:
                    nc.scalar.copy(out=dst, in_=grams[k][:])
                else:
                    nc.vector.tensor_copy(out=dst, in_=grams[k][:])
            split = offs[4]
            nc.sync.dma_start(out=gram_d[:, :split], in_=result[:, :split])
            nc.gpsimd.dma_start(out=gram_d[:, split:], in_=result[:, split:])
    nc.finalize()
    return nc


def _run(out, label, trace=False):
    from concourse.bass_utils import run_bass_kernel_spmd

    global _NC_CACHE
    d_full = (
        np.asarray(out).reshape(_B, _P).astype(np.float32)
        - np.asarray(label).reshape(_B, _P).astype(np.float32)
    ).astype(ml_dtypes.float8_e4m3fn)
    if _NC_CACHE is None:
        _NC_CACHE = _build_nc()
    in_maps = [{"d": d_full[i * _BSH : (i + 1) * _BSH]} for i in range(_NCORES)]
    res = run_bass_kernel_spmd(
        _NC_CACHE, in_maps, core_ids=list(range(_NCORES)), trace=trace
    )
    total = 0.0
    for r in res.results:
        total += float((_WFULL * r["gram"].astype(np.float64)).sum())
    loss = total / (_B * 8 * (_H - 2) * (_W - 2))
    return np.asarray(np.float32(loss)), res


def kernel(out, label):
    loss, _ = _run(out, label, trace=False)
    return loss


# revision 14
# speedup vs baseline: 1.0192x; 1.0192x over previous
"""Contrast-depth MSE loss on 8 Trainium2 NeuronCores.

Math: with d = out - label (per image, 32x32 grid flattened to p in [0,1024)),
the loss is an exact quadratic form

    loss = sum_{p,q} C[p,q] * G[p,q] / (B*8*30*30),
    G[p,q] = sum_img d[img,p] * d[img,q]

where C (the contrast-depth-conv quadratic form) is supported on the
diagonals q-p in {0, +-1, +-31, +-32, +-33}.  Each core computes banded
Gram blocks G[128k+r, 128k+c] (c in [0,161)) on the TensorEngine with
PSUM accumulation over its 2048-image shard; the host applies the C
weights to the diagonals and reduces across cores.

The host computes d = out - label exactly in fp32 and ships only d as
fp8e4m3 (2MB/core instead of 4.2MB) -- better numerics than the
quantize-then-subtract it replaces, half the DMA, and no on-device
vector work.  The PE runs in DoubleRow perf mode: each matmul contracts
two 128-image slots at once (lhsT/rhs carry 2 k-planes), which halves
both the instruction count and the weight-load traffic and doubles the
moving-stream rate.  Gram accumulation stays fp32 in PSUM.
"""

import numpy as np
import ml_dtypes

_B = 16384
_H = 32
_W = 32
_P = _H * _W  # 1024 pixels
_NCORES = 8
_BSH = _B // _NCORES  # 2048 images per core
_TILE = 128
_BAND = 161  # 128 + max diagonal offset (33)
_NSLOT = 16  # image-slots (128 images each)
_NPAIR = _NSLOT // 2
_NWARM = 20  # PE warmup matmuls (DVFS ramp) while DMA chunk 0 is in flight


def _block_ncols(k: int) -> int:
    return min(_BAND, _P - 128 * k)


_GRAM_COLS = sum(_block_ncols(k) for k in range(8))  # 7*161 + 128 = 1255


def _build_weights() -> np.ndarray:
    """[128, _GRAM_COLS] weights s.t. loss_sum = sum(W * gram_blocks)."""
    C = np.zeros((_P, _P), dtype=np.float64)
    offs = [(a, b) for a in range(3) for b in range(3) if (a, b) != (1, 1)]
    for a, b in offs:
        for i in range(_H - 2):
            for j in range(_W - 2):
                p = (i + a) * _W + (j + b)  # neighbor pixel
                q = (i + 1) * _W + (j + 1)  # center pixel
                C[p, p] += 1.0
                C[q, q] += 1.0
                C[p, q] -= 1.0
                C[q, p] -= 1.0
    W = np.zeros((_TILE, _GRAM_COLS), dtype=np.float64)
    off = 0
    for k in range(8):
        ncols = _block_ncols(k)
        for delta in (0, 1, 31, 32, 33):
            for r in range(_TILE):
                p = 128 * k + r
                q = p + delta
                c = r + delta
                if q >= _P or c >= ncols:
                    continue
                W[r, off + c] = C[p, q] * (1.0 if delta == 0 else 2.0)
        off += ncols
    return W


_WFULL = _build_weights()

_NC_CACHE = None


def _build_nc():
    import concourse.bacc as bacc
    import concourse.mybir as mybir
    import concourse.tile as tile

    nc = bacc.Bacc()
    d_d = nc.dram_tensor("d", [_BSH, _P], mybir.dt.float8e4, kind="ExternalInput")
    gram_d = nc.dram_tensor(
        "gram", [_TILE, _GRAM_COLS], mybir.dt.bfloat16, kind="ExternalOutput"
    )

    with tile.TileContext(nc) as tc:
        with (
            tc.tile_pool(name="buf", bufs=1) as buf_pool,
            tc.tile_pool(name="ps", bufs=1, space="PSUM") as psum_pool,
        ):
            grams = []
            offs = []
            off = 0
            for k in range(8):
                ncols = _block_ncols(k)
                grams.append(
                    psum_pool.tile(
                        [_TILE, ncols], mybir.dt.float32, tag=f"g{k}", name=f"g{k}"
                    )
                )
                offs.append(off)
                off += ncols

            # d[part, slot, pixel]: partition p of DMA chunk c holds images
            # 256c+2p (slot 2c) and 256c+2p+1 (slot 2c+1), 2KB contiguous.
            d = buf_pool.tile([_TILE, _NSLOT, _P], mybir.dt.float8e4, tag="d", name="d")
            result = buf_pool.tile(
                [_TILE, _GRAM_COLS], mybir.dt.bfloat16, tag="r", name="r"
            )
            dummy = buf_pool.tile(
                [_TILE, _BAND], mybir.dt.float8e4, tag="z", name="dummy"
            )

            # DMA triggers cost ~0.6us on the issuing queue and data lags the
            # trigger by ~1.5us (DGE->engine latency), so pair 0's two slots
            # are split into half-chunks triggered in parallel on sync and
            # gpsimd, and the remaining triggers alternate between the two
            # queues so no chunk ever gates the matmul stream.
            nc.gpsimd.memset(dummy[:], 0.0)
            issuers = [nc.sync, nc.gpsimd]
            for c in range(_NPAIR):
                n0 = c * 2 * _TILE
                issuers[c % 2].dma_start(
                    out=d[:, 2 * c : 2 * c + 2, :],
                    in_=d_d[n0 : n0 + 2 * _TILE, :],
                )

            # Warmup matmuls on a zeroed slab while the first DMA chunk is in
            # flight: keeps the PE busy so its DVFS p-state ramps toward
            # 2.4GHz before the real stream starts (cold PE issues at only
            # 1.2GHz).  Results land in g7, which pair 0 resets (start=True).
            for _ in range(_NWARM):
                nc.tensor.matmul(
                    grams[7][:, :128],
                    lhsT=dummy[:, 0:128],
                    rhs=dummy[:, 0:128],
                    start=True,
                    stop=True,
                )

            # one DoubleRow matmul per (pair, block): contracts both slots'
            # 128 images in a single instruction (2 k-planes).
            for c in range(_NPAIR):
                s0 = 2 * c
                for k in range(8):
                    ncols = _block_ncols(k)
                    nc.tensor.matmul(
                        grams[k][:, :ncols],
                        lhsT=d[:, s0 : s0 + 2, 128 * k : 128 * k + 128],
                        rhs=d[:, s0 : s0 + 2, 128 * k : 128 * k + ncols],
                        start=(c == 0),
                        stop=(c == _NPAIR - 1),
                        perf_mode=mybir.MatmulPerfMode.DoubleRow,
                    )

            # PSUM -> SBUF casts split across three engines (each block's copy
            # starts as soon as its stop-matmul retires), then one output DMA.
            for k in range(8):
                ncols = _block_ncols(k)
                dst = result[:, offs[k] : offs[k] + ncols]
                if k < 4:
                    nc.scalar.copy(out=dst, in_=grams[k][:])
                else:
                    nc.vector.tensor_copy(out=dst, in_=grams[k][:])
            # scalar triggers the slice it just copied (same-queue ordering,
            # no cross-engine sem hop); sync picks up the vector-cast slice.
            split = offs[4]
            nc.scalar.dma_start(out=gram_d[:, :split], in_=result[:, :split])
            nc.sync.dma_start(out=gram_d[:, split:], in_=result[:, split:])
    nc.finalize()
    return nc


def _run(out, label, trace=False):
    from concourse.bass_utils import run_bass_kernel_spmd

    global _NC_CACHE
    d_full = (
        np.asarray(out).reshape(_B, _P).astype(np.float32)
        - np.asarray(label).reshape(_B, _P).astype(np.float32)
    ).astype(ml_dtypes.float8_e4m3fn)
    if _NC_CACHE is None:
        _NC_CACHE = _build_nc()
    in_maps = [{"d": d_full[i * _BSH : (i + 1) * _BSH]} for i in range(_NCORES)]
    res = run_bass_kernel_spmd(
        _NC_CACHE, in_maps, core_ids=list(range(_NCORES)), trace=trace
    )
    total = 0.0
    for r in res.results:
        total += float((_WFULL * r["gram"].astype(np.float64)).sum())
    loss = total / (_B * 8 * (_H - 2) * (_W - 2))
    return np.asarray(np.float32(loss)), res


def kernel(out, label):
    loss, _ = _run(out, label, trace=False)
    return loss
